# revision 11
# baseline (speedup 1.0000x reference)
"""Trainium2 Bass kernel for nn_GatedAttention (linear attention with sigmoid
gate).

Strategy: shard the 16384 token rows across 8 cores (2048 each; cores 2b,2b+1
hold batch b). Per core, two phases:
  A: K,V projections (token-major) + per-head kv' = K^T [V|1] accumulated in
     PSUM over pairs of m-tiles (the ones column folds k_sum into kv'); kv
     matmuls are head-pair packed ([128,130] outputs, cross blocks ignored).
  -- pairwise AllReduce of kv' between the two cores sharing a batch --
  B: Q,G projections (feature-major), attention apply via one block-diagonal
     matmul per head pair, normalizer z broadcast via selector matmuls, gate,
     and the final output projection. Q/G for chunks 0-1 are emitted before
     anything that depends on the collective so the CC overlaps with them.
Host transposes x to feature-major and pre-transposes weights; output returns
feature-major per-core slabs that the host transposes back.
"""
import sys

sys.path.insert(0, "/opt/trn_rl_repo")

import numpy as np
import ml_dtypes

B, N, DIM = 4, 4096, 1024
HEADS, DH = 16, 64
SCALE = DH ** -0.5
N_CORES = 8
TPC = B * N // N_CORES      # 2048 tokens per core
NMT = TPC // 128            # 16 m-tiles (phase A)
CHUNK = 512
NCH = TPC // CHUNK          # 4 chunks (phase B)
CLAMP = 1e-6 / SCALE

DT_MODE = "bf16"            # "bf16" | "f32r" | "f32"
FP8_QK = False              # fp8-e4m3 DoubleRow for the Q,K projections
FP8_G = False               # fp8 for the G projection as well
XS, WS = 16.0, 256.0        # fp8 pre-scales for x and the weights
FSCL = 1.0 / (XS * WS)

_CACHE = {}


def _build(dt_mode=DT_MODE, reps=1):
    import concourse.bacc as bacc
    import concourse.bass as bass
    import concourse.tile as tile
    from concourse import mybir

    AF = mybir.ActivationFunctionType
    F32 = mybir.dt.float32
    DT = mybir.dt.bfloat16 if dt_mode == "bf16" else mybir.dt.float32

    def mm(ap):
        return ap.bitcast(mybir.dt.float32r) if dt_mode == "f32r" else ap

    ts = bass.ts

    nc = bacc.Bacc("TRN2", target_bir_lowering=False, debug=False,
                   num_devices=N_CORES)
    F8 = mybir.dt.float8e4
    xt = nc.dram_tensor("xt", [DIM, TPC], DT, kind="ExternalInput")
    w_in = {}
    fp8_w = ()
    if dt_mode == "bf16" and FP8_QK:
        fp8_w = ("wk", "wq", "wg") if FP8_G else ("wk", "wq")
        x8_d = nc.dram_tensor("x8", [DIM, TPC], F8, kind="ExternalInput")
    for nm in ("wk", "wv", "wq", "wg", "wo"):
        w_in[nm] = nc.dram_tensor(nm, [DIM, DIM],
                                  F8 if nm in fp8_w else DT,
                                  kind="ExternalInput")
    bg_d = nc.dram_tensor("bg", [DIM], F32, kind="ExternalInput")
    y_d = nc.dram_tensor("y", [DIM, TPC], F32, kind="ExternalOutput")
    cc_in = nc.dram_tensor("cc_in", [128, 8, 65], F32)
    cc_out = nc.dram_tensor("cc_out", [128, 8, 65], F32, addr_space="Shared")

    with tile.TileContext(nc, num_cores=N_CORES) as tc:
        with (
            tc.tile_pool(name="persist", bufs=1) as persist,
            tc.tile_pool(name="pb_big", bufs=2) as pb_big,
        ):
            X = persist.tile([128, 8, TPC], DT, tag="x")
            X8 = None
            if fp8_w:
                X8 = persist.tile([128, 8, TPC], F8, tag="x8", name="X8")
            wsb = {}
            for nm in ("wq", "wg", "wo"):
                wsb[nm] = persist.tile([128, 8, DIM],
                                       F8 if nm in fp8_w else DT,
                                       tag=nm, name=nm)
            bg_sb = persist.tile([128, 8], F32, tag="bg")
            sel_np = np.zeros((16, 8, 128), _np_dt(dt_mode))
            for p in range(8):
                sel_np[2 * p, p, 0:64] = 1.0
                sel_np[2 * p + 1, p, 64:128] = 1.0
            sel_d = nc.inline_tensor(sel_np, name="sel_const")
            sel = persist.tile([16, 8, 128], DT, tag="sel")

            for rep in range(reps):
                _phases(nc, tc, bass, mybir, AF, F32, DT, mm, ts, X, wsb,
                        bg_sb, sel, w_in, xt, bg_d, sel_d, cc_in, cc_out, y_d,
                        tc_pools=(persist, pb_big), load_persist=(rep == 0),
                        fp8_w=fp8_w, X8=X8,
                        x8_d=x8_d if fp8_w else None)
    nc.compile()
    return nc


def _phases(nc, tc, bass, mybir, AF, F32, DT, mm, ts, X, wsb, bg_sb, sel,
            w_in, xt, bg_d, sel_d, cc_in, cc_out, y_d, tc_pools, load_persist,
            fp8_w=(), X8=None, x8_d=None):
    persist, pb_big = tc_pools
    F8 = mybir.dt.float8e4
    DR = mybir.MatmulPerfMode.DoubleRow
    ALU = mybir.AluOpType

    # ---------------- phase A ----------------
    with (
        tc.tile_pool(name="pa_w", bufs=1) as pa_w,
        tc.tile_pool(name="pa_tmp", bufs=2) as pa_tmp,
        tc.tile_pool(name="pa_ps", bufs=2, space="PSUM") as pa_ps,
        tc.tile_pool(name="kv_ps", bufs=4, space="PSUM") as kv_pool,
    ):
        # DMA order: first-needed first.  X token-chunk 0 + wk + wv
        # interleaved per dim-block so the first K matmuls unblock after
        # ~0.5 MB instead of after the whole 14 MB preload.
        k_fp8 = "wk" in fp8_w
        wkv = {}
        wkv["wk"] = pa_w.tile([128, 8, DIM], F8 if k_fp8 else DT,
                              tag="wk", name="wk")
        wkv["wv"] = pa_w.tile([128, 8, DIM], DT, tag="wv", name="wv")
        for i in range(8):
            if load_persist and k_fp8:
                nc.sync.dma_start(out=X8[:, i, 0:512],
                                  in_=x8_d.ap()[ts(i, 128), 0:512])
            if load_persist:
                nc.sync.dma_start(out=X[:, i, 0:512],
                                  in_=xt.ap()[ts(i, 128), 0:512])
            nc.sync.dma_start(out=wkv["wk"][:, i, 0:512],
                              in_=w_in["wk"].ap()[ts(i, 128), 0:512])
            nc.sync.dma_start(out=wkv["wv"][:, i, 0:512],
                              in_=w_in["wv"].ap()[ts(i, 128), 0:512])
        for i in range(8):
            nc.sync.dma_start(out=wkv["wk"][:, i, 512:1024],
                              in_=w_in["wk"].ap()[ts(i, 128), 512:1024])
            nc.sync.dma_start(out=wkv["wv"][:, i, 512:1024],
                              in_=w_in["wv"].ap()[ts(i, 128), 512:1024])
        if load_persist:
            for c in range(1, 4):
                for i in range(8):
                    if k_fp8:
                        nc.sync.dma_start(out=X8[:, i, ts(c, 512)],
                                          in_=x8_d.ap()[ts(i, 128), ts(c, 512)])
                    nc.sync.dma_start(out=X[:, i, ts(c, 512)],
                                      in_=xt.ap()[ts(i, 128), ts(c, 512)])
            for nm in ("wq", "wg", "wo"):
                for i in range(8):
                    nc.sync.dma_start(out=wsb[nm][:, i, :],
                                      in_=w_in[nm].ap()[ts(i, 128), :])
            bg_ap = bg_d.ap()
            nc.sync.dma_start(
                out=bg_sb[:],
                in_=bass.AP(tensor=bg_ap.tensor, offset=0,
                            ap=[[1, 128], [128, 8]]),
            )
            nc.sync.dma_start(out=sel[:], in_=sel_d.ap())

        # kv accumulators live in PSUM for the whole phase: one bank per tile,
        # two head pairs each (cols 0:130 / 130:260).  Only the very first
        # matmul into a bank may carry start=True -- start zero-marks the
        # entire 2KB bank, so a second start would clobber the first pair's
        # accumulation state.
        kvts = [kv_pool.tile([128, 2, 130], F32, tag="kvw", name=f"kvw{t}")
                for t in range(4)]
        for mt in range(NMT):
            msl = ts(mt, 128)
            kps = pa_ps.tile([128, 1024], F32, tag="proj")
            if k_fp8:
                for i4 in range(4):
                    for o in range(2):
                        nc.tensor.matmul(
                            kps[:, ts(o, 512)],
                            X8[:, 2 * i4:2 * i4 + 2, msl],
                            wkv["wk"][:, 2 * i4:2 * i4 + 2, ts(o, 512)],
                            start=(i4 == 0), stop=(i4 == 3),
                            perf_mode=DR,
                        )
            else:
                for o in range(2):
                    for i in range(8):
                        nc.tensor.matmul(
                            kps[:, ts(o, 512)],
                            mm(X[:, i, msl]),
                            mm(wkv["wk"][:, i, ts(o, 512)]),
                            start=(i == 0), stop=(i == 7),
                        )
            kscl = FSCL if k_fp8 else 1.0
            # elu(x)+1 = max(x,0) + exp(min(x,0)); only Exp runs on ACT.
            m1 = pa_tmp.tile([128, 1024], DT, tag="m1")
            nc.vector.tensor_scalar_min(m1, kps, 0.0)
            e1 = pa_tmp.tile([128, 1024], DT, tag="e1")
            nc.scalar.activation(e1, m1, AF.Exp, scale=kscl)
            r1 = pa_tmp.tile([128, 1024], F32, tag="r1")
            if k_fp8:
                nc.vector.tensor_scalar(r1, kps, 0.0, kscl,
                                        ALU.max, ALU.mult)
            else:
                nc.vector.tensor_scalar_max(r1, kps, 0.0)
            ksb = pa_tmp.tile([128, 1024], DT, tag="ksb")
            nc.vector.tensor_add(ksb, r1, e1)

            vps = pa_ps.tile([128, 16, 64], F32, tag="proj")
            for o in range(2):
                for i in range(8):
                    nc.tensor.matmul(
                        vps[:, ts(o, 8), :],
                        mm(X[:, i, msl]),
                        mm(wkv["wv"][:, i, ts(o, 512)]),
                        start=(i == 0), stop=(i == 7),
                    )
            vp = pa_tmp.tile([128, 16, 65], DT, tag="vp")
            nc.vector.memset(vp[:, :, 64:65], 1.0)
            nc.vector.tensor_copy(vp[:, :, 0:64], vps[:, :, :])

            # kv accumulation: one matmul per head pair, accumulated in PSUM
            # over all m-tiles.
            for t in range(4):
                for q in range(2):
                    pp = 2 * t + q
                    nc.tensor.matmul(
                        kvts[t][:, q, :],
                        mm(ksb[:, ts(pp, 128)]),
                        mm(vp[:, 2 * pp:2 * pp + 2, :]),
                        start=(mt == 0 and q == 0), stop=(mt == NMT - 1),
                        skip_group_check=True,
                    )

        kv_sb = pa_tmp.tile([128, 8, 130], F32, tag="kv_sb", bufs=1,
                            name="kv_sb")
        for t in range(4):
            nc.vector.tensor_copy(kv_sb[:, 2 * t:2 * t + 2, :], kvts[t][:])
        # ship the diagonal blocks: [0:64, pp, 0:65] and [64:128, pp, 65:130]
        nc.sync.dma_start(out=cc_in.ap()[0:64, :, :],
                          in_=kv_sb[0:64, :, 0:65])
        nc.sync.dma_start(out=cc_in.ap()[64:128, :, :],
                          in_=kv_sb[64:128, :, 65:130])

    nc.gpsimd.collective_compute(
        "AllReduce",
        mybir.AluOpType.add,
        replica_groups=[[0, 1], [2, 3], [4, 5], [6, 7]],
        ins=[cc_in.ap().opt()],
        outs=[cc_out.ap().opt()],
    )

    # ---------------- phase B ----------------
    with (
        tc.tile_pool(name="pb_tmp", bufs=2) as pb_tmp,
        tc.tile_pool(name="pb_small", bufs=1) as pb_small,
        tc.tile_pool(name="ps_proj", bufs=2, space="PSUM") as ps_proj,
        tc.tile_pool(name="ps_misc", bufs=4, space="PSUM") as ps_misc,
        tc.tile_pool(name="ps_y", bufs=2, space="PSUM") as ps_y,
    ):
        qsb = [None] * NCH
        gsb = [None] * NCH

        def proj_q(ch):
            csl = ts(ch, CHUNK)
            qsb[ch] = pb_big.tile([128, 8, CHUNK], DT, tag="qsb", name="qsb",
                                  bufs=3)
            q_fp8 = "wq" in fp8_w
            for p in range(8):
                qps = ps_proj.tile([128, CHUNK], F32, tag="proj")
                if q_fp8:
                    for i4 in range(4):
                        nc.tensor.matmul(
                            qps,
                            wsb["wq"][:, 2 * i4:2 * i4 + 2, ts(p, 128)],
                            X8[:, 2 * i4:2 * i4 + 2, csl],
                            start=(i4 == 0), stop=(i4 == 3),
                            perf_mode=DR,
                        )
                else:
                    for i in range(8):
                        nc.tensor.matmul(
                            qps, mm(wsb["wq"][:, i, ts(p, 128)]),
                            mm(X[:, i, csl]),
                            start=(i == 0), stop=(i == 7),
                        )
                qscl = FSCL if q_fp8 else 1.0
                m1 = pb_tmp.tile([128, CHUNK], F32, tag="bm1")
                nc.vector.tensor_scalar_min(m1, qps, 0.0)
                e1 = pb_tmp.tile([128, CHUNK], F32, tag="be1")
                nc.scalar.activation(e1, m1, AF.Exp, scale=qscl)
                r1 = pb_tmp.tile([128, CHUNK], F32, tag="br1")
                if q_fp8:
                    nc.vector.tensor_scalar(r1, qps, 0.0, qscl,
                                            ALU.max, ALU.mult)
                else:
                    nc.vector.tensor_scalar_max(r1, qps, 0.0)
                nc.vector.tensor_add(qsb[ch][:, p, :], r1, e1)

        def proj_g(ch):
            csl = ts(ch, CHUNK)
            gsb[ch] = pb_big.tile([128, 8, CHUNK], DT, tag="gsb", name="gsb",
                                  bufs=2)
            g_fp8 = "wg" in fp8_w
            for p in range(8):
                gps = ps_proj.tile([128, CHUNK], F32, tag="proj")
                if g_fp8:
                    for i4 in range(4):
                        nc.tensor.matmul(
                            gps,
                            wsb["wg"][:, 2 * i4:2 * i4 + 2, ts(p, 128)],
                            X8[:, 2 * i4:2 * i4 + 2, csl],
                            start=(i4 == 0), stop=(i4 == 3),
                            perf_mode=DR,
                        )
                else:
                    for i in range(8):
                        nc.tensor.matmul(
                            gps, mm(wsb["wg"][:, i, ts(p, 128)]),
                            mm(X[:, i, csl]),
                            start=(i == 0), stop=(i == 7),
                        )
                nc.scalar.activation(gsb[ch][:, p, :], gps, AF.Sigmoid,
                                     scale=(FSCL if g_fp8 else 1.0),
                                     bias=bg_sb[:, p:p + 1])

        def attn_out(ch, kvd, ksd):
            csl = ts(ch, CHUNK)
            qkps = ps_misc.tile([16, CHUNK], F32, tag="misc")
            for p in range(8):
                nc.tensor.matmul(
                    qkps, mm(ksd[:, p, :]), mm(qsb[ch][:, p, :]),
                    start=(p == 0), stop=(p == 7),
                    skip_group_check=True,
                )
            zq = pb_tmp.tile([16, CHUNK], F32, tag="zq")
            nc.vector.tensor_scalar_max(zq, qkps, CLAMP)
            zr = pb_tmp.tile([16, CHUNK], F32, tag="zr")
            nc.vector.reciprocal(zr, zq)
            zqr = pb_tmp.tile([16, CHUNK], DT, tag="zqr")
            nc.vector.tensor_copy(zqr, zr)

            asb = pb_big.tile([128, 8, CHUNK], DT, tag="asb")
            for p in range(8):
                zbps = ps_misc.tile([128, CHUNK], F32, tag="misc")
                nc.tensor.matmul(zbps, mm(sel[:, p, :]), mm(zqr),
                                 start=True, stop=True)
                ops_ = ps_misc.tile([128, CHUNK], F32, tag="misc")
                nc.tensor.matmul(ops_, mm(kvd[:, p, :]),
                                 mm(qsb[ch][:, p, :]),
                                 start=True, stop=True)
                t1 = pb_tmp.tile([128, CHUNK], F32, tag="bt1")
                nc.vector.tensor_mul(t1, ops_, gsb[ch][:, p, :])
                nc.vector.tensor_mul(asb[:, p, :], t1, zbps)

            for d in range(8):
                yps = ps_y.tile([128, CHUNK], F32, tag="y")
                for fi in range(8):
                    nc.tensor.matmul(
                        yps, mm(wsb["wo"][:, fi, ts(d, 128)]),
                        mm(asb[:, fi, :]),
                        start=(fi == 0), stop=(fi == 7),
                    )
                ysb = pb_tmp.tile([128, CHUNK], F32, tag="ysb")
                nc.scalar.copy(ysb, yps)
                nc.sync.dma_start(out=y_d.ap()[ts(d, 128), csl],
                                  in_=ysb[:])

        # Q/G for chunks 0-1 plus Q2 are independent of the collective:
        # emit them first so the PE fills the AllReduce window.
        proj_q(0)
        proj_g(0)
        proj_q(1)
        proj_g(1)
        proj_q(2)

        kvf = pb_small.tile([128, 8, 65], F32, tag="kvf")
        nc.sync.dma_start(out=kvf[:], in_=cc_out.ap()[:, :, :])
        # block-diagonal per-pair kv (bf16) for the single-matmul apply
        kvd = pb_small.tile([128, 8, 128], DT, tag="kvd")
        nc.vector.memset(kvd[:], 0.0)
        for p in range(8):
            nc.vector.tensor_copy(kvd[0:64, p, 0:64], kvf[0:64, p, 0:64])
            nc.vector.tensor_copy(kvd[64:128, p, 64:128],
                                  kvf[64:128, p, 0:64])
        ksd = pb_small.tile([128, 8, 16], DT, tag="ksd")
        nc.vector.memset(ksd[:], 0.0)
        for p in range(8):
            nc.scalar.activation(ksd[0:64, p, 2 * p:2 * p + 1],
                                 kvf[0:64, p, 64:65],
                                 AF.Copy, scale=1.0 / SCALE)
            nc.scalar.activation(ksd[64:128, p, 2 * p + 1:2 * p + 2],
                                 kvf[64:128, p, 64:65],
                                 AF.Copy, scale=1.0 / SCALE)

        attn_out(0, kvd, ksd)
        proj_g(2)
        attn_out(1, kvd, ksd)
        proj_q(3)
        attn_out(2, kvd, ksd)
        proj_g(3)
        attn_out(3, kvd, ksd)


def _np_dt(dt_mode):
    return ml_dtypes.bfloat16 if dt_mode == "bf16" else np.float32


def _q8(a, scale):
    f8 = ml_dtypes.float8_e4m3
    return np.clip(np.asarray(a, np.float32) * scale,
                   -240.0, 240.0).astype(f8)


def prep_inputs(x, Wq, Wk, Wv, Wg, bg, Wo, dt_mode=DT_MODE):
    npdt = _np_dt(dt_mode)
    fp8_w = ()
    if dt_mode == "bf16" and FP8_QK:
        fp8_w = ("wq", "wk", "wg") if FP8_G else ("wq", "wk")
    x_f = np.ascontiguousarray(np.asarray(x, np.float32).reshape(B * N, DIM))
    w_t = {}
    for nm, W in (("wq", Wq), ("wk", Wk), ("wv", Wv), ("wg", Wg),
                  ("wo", Wo)):
        wT = np.ascontiguousarray(np.asarray(W, np.float32).T)
        if nm in fp8_w:
            w_t[nm] = np.ascontiguousarray(_q8(wT, WS))
        else:
            w_t[nm] = wT.astype(npdt)
    bg_f = np.ascontiguousarray(np.asarray(bg, np.float32))
    in_maps = []
    for c in range(N_CORES):
        xt_ct = np.ascontiguousarray(x_f[c * TPC:(c + 1) * TPC].T)
        m = {"xt": xt_ct.astype(npdt), "bg": bg_f}
        if fp8_w:
            m["x8"] = np.ascontiguousarray(_q8(xt_ct, XS))
        m.update(w_t)
        in_maps.append(m)
    return in_maps


def unshard_output(y_parts):
    out = np.empty((B * N, DIM), np.float32)
    for c in range(N_CORES):
        out[c * TPC:(c + 1) * TPC] = np.asarray(y_parts[c]).T
    return out.reshape(B, N, DIM)


def get_nc(dt_mode=DT_MODE):
    key = ("nc", dt_mode)
    if key not in _CACHE:
        _CACHE[key] = _build(dt_mode)
    return _CACHE[key]


def kernel(x, Wq, Wk, Wv, Wg, bg, Wo):
    from concourse.bass_utils import run_bass_kernel_spmd

    nc = get_nc()
    in_maps = prep_inputs(x, Wq, Wk, Wv, Wg, bg, Wo)
    res = run_bass_kernel_spmd(nc, in_maps, core_ids=list(range(N_CORES)))
    return unshard_output([res.results[c]["y"] for c in range(N_CORES)])


# revision 13
# speedup vs baseline: 1.1163x; 1.1163x over previous
"""Trainium2 Bass kernel for nn_GatedAttention (linear attention with sigmoid
gate).

Strategy: shard the 16384 token rows across 8 cores (2048 each; cores 2b,2b+1
hold batch b). Per core, two phases:
  A: K,V projections (token-major) + per-head kv' = K^T [V|1] accumulated in
     PSUM over pairs of m-tiles (the ones column folds k_sum into kv'); kv
     matmuls are head-pair packed ([128,130] outputs, cross blocks ignored).
  -- pairwise AllReduce of kv' between the two cores sharing a batch --
  B: Q,G projections (feature-major), attention apply via one block-diagonal
     matmul per head pair, normalizer z broadcast via selector matmuls, gate,
     and the final output projection. Q/G for chunks 0-1 are emitted before
     anything that depends on the collective so the CC overlaps with them.
Host transposes x to feature-major and pre-transposes weights; output returns
feature-major per-core slabs that the host transposes back.
"""
import sys

sys.path.insert(0, "/opt/trn_rl_repo")

import numpy as np
import ml_dtypes

B, N, DIM = 4, 4096, 1024
HEADS, DH = 16, 64
SCALE = DH ** -0.5
N_CORES = 8
TPC = B * N // N_CORES      # 2048 tokens per core
NMT = TPC // 128            # 16 m-tiles (phase A)
CHUNK = 512
NCH = TPC // CHUNK          # 4 chunks (phase B)
CLAMP = 1e-6 / SCALE

DT_MODE = "bf16"            # "bf16" | "f32r" | "f32"
FP8_QK = False              # fp8-e4m3 DoubleRow for the Q,K projections
FP8_G = False               # fp8 for the G projection as well
XS, WS = 16.0, 256.0        # fp8 pre-scales for x and the weights
FSCL = 1.0 / (XS * WS)

_CACHE = {}


def _build(dt_mode=DT_MODE, reps=1):
    import concourse.bacc as bacc
    import concourse.bass as bass
    import concourse.tile as tile
    from concourse import mybir

    AF = mybir.ActivationFunctionType
    F32 = mybir.dt.float32
    DT = mybir.dt.bfloat16 if dt_mode == "bf16" else mybir.dt.float32

    def mm(ap):
        return ap.bitcast(mybir.dt.float32r) if dt_mode == "f32r" else ap

    ts = bass.ts

    nc = bacc.Bacc("TRN2", target_bir_lowering=False, debug=False,
                   num_devices=N_CORES)
    F8 = mybir.dt.float8e4
    xt = nc.dram_tensor("xt", [DIM, TPC], DT, kind="ExternalInput")
    w_in = {}
    fp8_w = ()
    if dt_mode == "bf16" and FP8_QK:
        fp8_w = ("wk", "wq", "wg") if FP8_G else ("wk", "wq")
        x8_d = nc.dram_tensor("x8", [DIM, TPC], F8, kind="ExternalInput")
    for nm in ("wk", "wv", "wq", "wg", "wo"):
        w_in[nm] = nc.dram_tensor(nm, [DIM, DIM],
                                  F8 if nm in fp8_w else DT,
                                  kind="ExternalInput")
    bg_d = nc.dram_tensor("bg", [DIM], F32, kind="ExternalInput")
    y_d = nc.dram_tensor("y", [DIM, TPC], F32, kind="ExternalOutput")
    cc_in = nc.dram_tensor("cc_in", [128, 8, 65], F32)
    cc_out = nc.dram_tensor("cc_out", [128, 8, 65], F32)

    with tile.TileContext(nc, num_cores=N_CORES) as tc:
        with (
            tc.tile_pool(name="persist", bufs=1) as persist,
            tc.tile_pool(name="pb_big", bufs=2) as pb_big,
        ):
            X = persist.tile([128, 8, TPC], DT, tag="x")
            X8 = None
            if fp8_w:
                X8 = persist.tile([128, 8, TPC], F8, tag="x8", name="X8")
            wsb = {}
            for nm in ("wq", "wg", "wo"):
                wsb[nm] = persist.tile([128, 8, DIM],
                                       F8 if nm in fp8_w else DT,
                                       tag=nm, name=nm)
            bg_sb = persist.tile([128, 8], F32, tag="bg")
            sel_np = np.zeros((16, 8, 128), _np_dt(dt_mode))
            for p in range(8):
                sel_np[2 * p, p, 0:64] = 1.0
                sel_np[2 * p + 1, p, 64:128] = 1.0
            sel_d = nc.inline_tensor(sel_np, name="sel_const")
            sel = persist.tile([16, 8, 128], DT, tag="sel")

            for rep in range(reps):
                _phases(nc, tc, bass, mybir, AF, F32, DT, mm, ts, X, wsb,
                        bg_sb, sel, w_in, xt, bg_d, sel_d, cc_in, cc_out, y_d,
                        tc_pools=(persist, pb_big), load_persist=(rep == 0),
                        fp8_w=fp8_w, X8=X8,
                        x8_d=x8_d if fp8_w else None)
    nc.compile()
    return nc


def _phases(nc, tc, bass, mybir, AF, F32, DT, mm, ts, X, wsb, bg_sb, sel,
            w_in, xt, bg_d, sel_d, cc_in, cc_out, y_d, tc_pools, load_persist,
            fp8_w=(), X8=None, x8_d=None):
    persist, pb_big = tc_pools
    F8 = mybir.dt.float8e4
    DR = mybir.MatmulPerfMode.DoubleRow
    ALU = mybir.AluOpType

    # ---------------- phase A ----------------
    with (
        tc.tile_pool(name="pa_w", bufs=1) as pa_w,
        tc.tile_pool(name="pa_tmp", bufs=2) as pa_tmp,
        tc.tile_pool(name="pa_ps", bufs=2, space="PSUM") as pa_ps,
        tc.tile_pool(name="kv_ps", bufs=4, space="PSUM") as kv_pool,
    ):
        # DMA order: first-needed first.  X token-chunk 0 + wk + wv
        # interleaved per dim-block so the first K matmuls unblock after
        # ~0.5 MB instead of after the whole 14 MB preload.
        k_fp8 = "wk" in fp8_w
        wkv = {}
        wkv["wk"] = pa_w.tile([128, 8, DIM], F8 if k_fp8 else DT,
                              tag="wk", name="wk")
        wkv["wv"] = pa_w.tile([128, 8, DIM], DT, tag="wv", name="wv")
        for i in range(8):
            if load_persist and k_fp8:
                nc.sync.dma_start(out=X8[:, i, 0:512],
                                  in_=x8_d.ap()[ts(i, 128), 0:512])
            if load_persist:
                nc.sync.dma_start(out=X[:, i, 0:512],
                                  in_=xt.ap()[ts(i, 128), 0:512])
            nc.sync.dma_start(out=wkv["wk"][:, i, :],
                              in_=w_in["wk"].ap()[ts(i, 128), :])
            nc.sync.dma_start(out=wkv["wv"][:, i, :],
                              in_=w_in["wv"].ap()[ts(i, 128), :])
        if load_persist:
            for c in range(1, 4):
                for i in range(8):
                    if k_fp8:
                        nc.sync.dma_start(out=X8[:, i, ts(c, 512)],
                                          in_=x8_d.ap()[ts(i, 128), ts(c, 512)])
                    nc.sync.dma_start(out=X[:, i, ts(c, 512)],
                                      in_=xt.ap()[ts(i, 128), ts(c, 512)])
            for nm in ("wq", "wg", "wo"):
                for i in range(8):
                    nc.sync.dma_start(out=wsb[nm][:, i, :],
                                      in_=w_in[nm].ap()[ts(i, 128), :])
            bg_ap = bg_d.ap()
            nc.sync.dma_start(
                out=bg_sb[:],
                in_=bass.AP(tensor=bg_ap.tensor, offset=0,
                            ap=[[1, 128], [128, 8]]),
            )
            nc.sync.dma_start(out=sel[:], in_=sel_d.ap())

        # kv accumulators live in PSUM for the whole phase: one bank per tile,
        # two head pairs each (cols 0:130 / 130:260).  Only the very first
        # matmul into a bank may carry start=True -- start zero-marks the
        # entire 2KB bank, so a second start would clobber the first pair's
        # accumulation state.
        kvts = [kv_pool.tile([128, 2, 130], F32, tag="kvw", name=f"kvw{t}")
                for t in range(4)]
        for mt in range(NMT):
            msl = ts(mt, 128)
            kps = pa_ps.tile([128, 1024], F32, tag="proj")
            if k_fp8:
                for i4 in range(4):
                    for o in range(2):
                        nc.tensor.matmul(
                            kps[:, ts(o, 512)],
                            X8[:, 2 * i4:2 * i4 + 2, msl],
                            wkv["wk"][:, 2 * i4:2 * i4 + 2, ts(o, 512)],
                            start=(i4 == 0), stop=(i4 == 3),
                            perf_mode=DR,
                        )
            else:
                for i in range(8):
                    for o in range(2):
                        nc.tensor.matmul(
                            kps[:, ts(o, 512)],
                            mm(X[:, i, msl]),
                            mm(wkv["wk"][:, i, ts(o, 512)]),
                            start=(i == 0), stop=(i == 7),
                        )
            kscl = FSCL if k_fp8 else 1.0
            # elu(x)+1 = max(x,0) + exp(min(x,0)); only Exp runs on ACT.
            m1 = pa_tmp.tile([128, 1024], DT, tag="m1")
            nc.vector.tensor_scalar_min(m1, kps, 0.0)
            e1 = pa_tmp.tile([128, 1024], DT, tag="e1")
            nc.scalar.activation(e1, m1, AF.Exp, scale=kscl)
            r1 = pa_tmp.tile([128, 1024], F32, tag="r1")
            if k_fp8:
                nc.vector.tensor_scalar(r1, kps, 0.0, kscl,
                                        ALU.max, ALU.mult)
            else:
                nc.vector.tensor_scalar_max(r1, kps, 0.0)
            ksb = pa_tmp.tile([128, 1024], DT, tag="ksb")
            nc.vector.tensor_add(ksb, r1, e1)

            vps = pa_ps.tile([128, 16, 64], F32, tag="proj")
            for i in range(8):
                for o in range(2):
                    nc.tensor.matmul(
                        vps[:, ts(o, 8), :],
                        mm(X[:, i, msl]),
                        mm(wkv["wv"][:, i, ts(o, 512)]),
                        start=(i == 0), stop=(i == 7),
                    )
            vp = pa_tmp.tile([128, 16, 65], DT, tag="vp")
            nc.vector.memset(vp[:, :, 64:65], 1.0)
            nc.vector.tensor_copy(vp[:, :, 0:64], vps[:, :, :])

            # kv accumulation: one matmul per head pair, accumulated in PSUM
            # over all m-tiles.
            for t in range(4):
                for q in range(2):
                    pp = 2 * t + q
                    nc.tensor.matmul(
                        kvts[t][:, q, :],
                        mm(ksb[:, ts(pp, 128)]),
                        mm(vp[:, 2 * pp:2 * pp + 2, :]),
                        start=(mt == 0 and q == 0), stop=(mt == NMT - 1),
                        skip_group_check=True,
                    )

        kv_sb = pa_tmp.tile([128, 8, 130], F32, tag="kv_sb", bufs=1,
                            name="kv_sb")
        for t in range(4):
            nc.vector.tensor_copy(kv_sb[:, 2 * t:2 * t + 2, :], kvts[t][:])
        # ship the diagonal blocks: [0:64, pp, 0:65] and [64:128, pp, 65:130]
        nc.sync.dma_start(out=cc_in.ap()[0:64, :, :],
                          in_=kv_sb[0:64, :, 0:65])
        nc.sync.dma_start(out=cc_in.ap()[64:128, :, :],
                          in_=kv_sb[64:128, :, 65:130])

    nc.gpsimd.collective_compute(
        "AllReduce",
        mybir.AluOpType.add,
        replica_groups=[[0, 1], [2, 3], [4, 5], [6, 7]],
        ins=[cc_in.ap().opt()],
        outs=[cc_out.ap().opt()],
    )

    # ---------------- phase B ----------------
    with (
        tc.tile_pool(name="pb_tmp", bufs=2) as pb_tmp,
        tc.tile_pool(name="pb_small", bufs=1) as pb_small,
        tc.tile_pool(name="ps_proj", bufs=2, space="PSUM") as ps_proj,
        tc.tile_pool(name="ps_misc", bufs=4, space="PSUM") as ps_misc,
        tc.tile_pool(name="ps_y", bufs=2, space="PSUM") as ps_y,
    ):
        qsb = [None] * NCH
        gsb = [None] * NCH

        def proj_q(ch):
            csl = ts(ch, CHUNK)
            qsb[ch] = pb_big.tile([128, 8, CHUNK], DT, tag="qsb", name="qsb",
                                  bufs=3)
            q_fp8 = "wq" in fp8_w
            for p in range(8):
                qps = ps_proj.tile([128, CHUNK], F32, tag="proj")
                if q_fp8:
                    for i4 in range(4):
                        nc.tensor.matmul(
                            qps,
                            wsb["wq"][:, 2 * i4:2 * i4 + 2, ts(p, 128)],
                            X8[:, 2 * i4:2 * i4 + 2, csl],
                            start=(i4 == 0), stop=(i4 == 3),
                            perf_mode=DR,
                        )
                else:
                    for i in range(8):
                        nc.tensor.matmul(
                            qps, mm(wsb["wq"][:, i, ts(p, 128)]),
                            mm(X[:, i, csl]),
                            start=(i == 0), stop=(i == 7),
                        )
                qscl = FSCL if q_fp8 else 1.0
                m1 = pb_tmp.tile([128, CHUNK], F32, tag="bm1")
                nc.vector.tensor_scalar_min(m1, qps, 0.0)
                e1 = pb_tmp.tile([128, CHUNK], F32, tag="be1")
                nc.scalar.activation(e1, m1, AF.Exp, scale=qscl)
                r1 = pb_tmp.tile([128, CHUNK], F32, tag="br1")
                if q_fp8:
                    nc.vector.tensor_scalar(r1, qps, 0.0, qscl,
                                            ALU.max, ALU.mult)
                else:
                    nc.vector.tensor_scalar_max(r1, qps, 0.0)
                nc.vector.tensor_add(qsb[ch][:, p, :], r1, e1)

        def proj_g(ch):
            csl = ts(ch, CHUNK)
            gsb[ch] = pb_big.tile([128, 8, CHUNK], DT, tag="gsb", name="gsb",
                                  bufs=2)
            g_fp8 = "wg" in fp8_w
            for p in range(8):
                gps = ps_proj.tile([128, CHUNK], F32, tag="proj")
                if g_fp8:
                    for i4 in range(4):
                        nc.tensor.matmul(
                            gps,
                            wsb["wg"][:, 2 * i4:2 * i4 + 2, ts(p, 128)],
                            X8[:, 2 * i4:2 * i4 + 2, csl],
                            start=(i4 == 0), stop=(i4 == 3),
                            perf_mode=DR,
                        )
                else:
                    for i in range(8):
                        nc.tensor.matmul(
                            gps, mm(wsb["wg"][:, i, ts(p, 128)]),
                            mm(X[:, i, csl]),
                            start=(i == 0), stop=(i == 7),
                        )
                nc.scalar.activation(gsb[ch][:, p, :], gps, AF.Sigmoid,
                                     scale=(FSCL if g_fp8 else 1.0),
                                     bias=bg_sb[:, p:p + 1])

        def attn_out(ch, kvd, ksd):
            csl = ts(ch, CHUNK)
            qkps = ps_misc.tile([16, CHUNK], F32, tag="misc")
            for p in range(8):
                nc.tensor.matmul(
                    qkps, mm(ksd[:, p, :]), mm(qsb[ch][:, p, :]),
                    start=(p == 0), stop=(p == 7),
                    skip_group_check=True,
                )
            zq = pb_tmp.tile([16, CHUNK], F32, tag="zq")
            nc.vector.tensor_scalar_max(zq, qkps, CLAMP)
            zr = pb_tmp.tile([16, CHUNK], F32, tag="zr")
            nc.vector.reciprocal(zr, zq)
            zqr = pb_tmp.tile([16, CHUNK], DT, tag="zqr")
            nc.vector.tensor_copy(zqr, zr)

            asb = pb_big.tile([128, 8, CHUNK], DT, tag="asb")
            for p in range(8):
                zbps = ps_misc.tile([128, CHUNK], F32, tag="misc")
                nc.tensor.matmul(zbps, mm(sel[:, p, :]), mm(zqr),
                                 start=True, stop=True)
                ops_ = ps_misc.tile([128, CHUNK], F32, tag="misc")
                nc.tensor.matmul(ops_, mm(kvd[:, p, :]),
                                 mm(qsb[ch][:, p, :]),
                                 start=True, stop=True)
                t1 = pb_tmp.tile([128, CHUNK], F32, tag="bt1")
                nc.vector.tensor_mul(t1, ops_, gsb[ch][:, p, :])
                nc.vector.tensor_mul(asb[:, p, :], t1, zbps)

            for d in range(8):
                yps = ps_y.tile([128, CHUNK], F32, tag="y")
                for fi in range(8):
                    nc.tensor.matmul(
                        yps, mm(wsb["wo"][:, fi, ts(d, 128)]),
                        mm(asb[:, fi, :]),
                        start=(fi == 0), stop=(fi == 7),
                    )
                ysb = pb_tmp.tile([128, CHUNK], F32, tag="ysb")
                nc.scalar.copy(ysb, yps)
                nc.sync.dma_start(out=y_d.ap()[ts(d, 128), csl],
                                  in_=ysb[:])

        # Q/G for chunks 0-1 plus Q2 are independent of the collective:
        # emit them first so the PE fills the AllReduce window.
        proj_q(0)
        proj_g(0)
        proj_q(1)
        proj_g(1)
        proj_q(2)

        kvf = pb_small.tile([128, 8, 65], F32, tag="kvf")
        nc.sync.dma_start(out=kvf[:], in_=cc_out.ap()[:, :, :])
        # block-diagonal per-pair kv (bf16) for the single-matmul apply
        kvd = pb_small.tile([128, 8, 128], DT, tag="kvd")
        nc.vector.memset(kvd[:], 0.0)
        for p in range(8):
            nc.vector.tensor_copy(kvd[0:64, p, 0:64], kvf[0:64, p, 0:64])
            nc.vector.tensor_copy(kvd[64:128, p, 64:128],
                                  kvf[64:128, p, 0:64])
        ksd = pb_small.tile([128, 8, 16], DT, tag="ksd")
        nc.vector.memset(ksd[:], 0.0)
        for p in range(8):
            nc.scalar.activation(ksd[0:64, p, 2 * p:2 * p + 1],
                                 kvf[0:64, p, 64:65],
                                 AF.Copy, scale=1.0 / SCALE)
            nc.scalar.activation(ksd[64:128, p, 2 * p + 1:2 * p + 2],
                                 kvf[64:128, p, 64:65],
                                 AF.Copy, scale=1.0 / SCALE)

        attn_out(0, kvd, ksd)
        proj_g(2)
        attn_out(1, kvd, ksd)
        proj_q(3)
        attn_out(2, kvd, ksd)
        proj_g(3)
        attn_out(3, kvd, ksd)


def _np_dt(dt_mode):
    return ml_dtypes.bfloat16 if dt_mode == "bf16" else np.float32


def _q8(a, scale):
    f8 = ml_dtypes.float8_e4m3
    return np.clip(np.asarray(a, np.float32) * scale,
                   -240.0, 240.0).astype(f8)


def prep_inputs(x, Wq, Wk, Wv, Wg, bg, Wo, dt_mode=DT_MODE):
    npdt = _np_dt(dt_mode)
    fp8_w = ()
    if dt_mode == "bf16" and FP8_QK:
        fp8_w = ("wq", "wk", "wg") if FP8_G else ("wq", "wk")
    x_f = np.ascontiguousarray(np.asarray(x, np.float32).reshape(B * N, DIM))
    w_t = {}
    for nm, W in (("wq", Wq), ("wk", Wk), ("wv", Wv), ("wg", Wg),
                  ("wo", Wo)):
        wT = np.ascontiguousarray(np.asarray(W, np.float32).T)
        if nm in fp8_w:
            w_t[nm] = np.ascontiguousarray(_q8(wT, WS))
        else:
            w_t[nm] = wT.astype(npdt)
    bg_f = np.ascontiguousarray(np.asarray(bg, np.float32))
    in_maps = []
    for c in range(N_CORES):
        xt_ct = np.ascontiguousarray(x_f[c * TPC:(c + 1) * TPC].T)
        m = {"xt": xt_ct.astype(npdt), "bg": bg_f}
        if fp8_w:
            m["x8"] = np.ascontiguousarray(_q8(xt_ct, XS))
        m.update(w_t)
        in_maps.append(m)
    return in_maps


def unshard_output(y_parts):
    out = np.empty((B * N, DIM), np.float32)
    for c in range(N_CORES):
        out[c * TPC:(c + 1) * TPC] = np.asarray(y_parts[c]).T
    return out.reshape(B, N, DIM)


def get_nc(dt_mode=DT_MODE):
    key = ("nc", dt_mode)
    if key not in _CACHE:
        _CACHE[key] = _build(dt_mode)
    return _CACHE[key]


def kernel(x, Wq, Wk, Wv, Wg, bg, Wo):
    from concourse.bass_utils import run_bass_kernel_spmd

    nc = get_nc()
    in_maps = prep_inputs(x, Wq, Wk, Wv, Wg, bg, Wo)
    res = run_bass_kernel_spmd(nc, in_maps, core_ids=list(range(N_CORES)))
    return unshard_output([res.results[c]["y"] for c in range(N_CORES)])
